# revision 24
# baseline (speedup 1.0000x reference)
"""Trainium2 Bass kernel for BuildVolume2d (stereo cost volume, L1 over channels).

cost[b, d, h, w] = sum_c |feat_l[b,c,h,w] - feat_r[b,c,h,4w-d]|   (feat_r zero-padded left)

Sharding: batch B=8 -> 8 NeuronCores (data parallel, one sample per core).

v3 design (per core, sample b; h-groups of 4 rows, partitions p = 32h + c):
  - Host pre-bakes fp16 layouts (no on-chip casts / phase splits):
      l16[(h c), w]                  = l[c,h,w]
      rstack[(h c), 524*s + 12 + m]  = r[c,h,4m-s]  (zero where 4m-s < 0 or m < 0)
    For d = 4q+s:  diff[p, q, s, w] = l16[p, w] - rstack[p, 524*s + 12 + w - q]
    i.e. in0 is a pure broadcast over (q, s) and in1 has q-stride -1: big
    multi-q subtracts are single instructions with a dense [128, n*4*512] out.
  - |x| in place, split across engines by q-range. Engine split tuned against
    the HW trace: the chip power-throttles (~0.7 util) and SBUF ports are
    contended, so ACT (1x but otherwise idle) carries most of the abs;
    DVE (4x tensor_scalar via int16 sign-bit clear) takes a little; GpSimd
    subtracts the tail q's.
  - Channel reduce: one matmul per disparity d, moving = dif[:, 512d:512d+512],
    8 one-hot stationaries on8[j][k, 4j + k//32]; 8 matmuls accumulate per
    32-row PSUM block; PSUM rows 4*dd + h, col block cb (d = 24cb + dd).
  - Drain: ACT copy PSUM[96,1024] f32 -> SBUF fp16.
  - Output DMA'd as fp16 [48,256,512] (2D sbuf -> 3D dram AP); host casts f32.
"""
import sys
sys.path.insert(0, '/opt/trn_rl_repo')

import numpy as np
import concourse.bass as bass
import concourse.tile as tile
from concourse import bacc, mybir
from concourse.ap import AP
from concourse.bass_utils import run_bass_kernel_spmd

# ---- problem constants (hardcoded per spec) ----
B, C, H, W = 8, 32, 256, 512
W4 = 4 * W
D = 48                     # maxdisp
N_CORES = 8
HG = 4                     # h rows per group
N_HG = H // HG             # 64
NQ = 12                    # d = 4q + s
RW = 2096                  # rstack row width (4 phase blocks of 524)
FQ = 4 * W                 # 2048 diff elems per q

# ---- engine split knobs (tuned against HW traces) ----
SQ = 8                     # DVE subtracts q in [0, SQ); GpSimd does [SQ, 12)
ABS_ACT_Q = 10             # ACT abs on q in [0, ABS_ACT_Q)
ABS_DVE_Q = 12             # DVE abs on [ABS_ACT_Q, ABS_DVE_Q); GpSimd rest
SUB_CHUNK = 3              # DVE sub emitted in q-chunks of this size
ABS_CHUNK = 2              # ACT abs emitted in q-chunks of this size
DIFF_BUFS = 2
PSUM_BUFS = 2
OOP_ABS = True             # ACT abs writes separate tiles (less tile contention)

f32 = mybir.dt.float32
f16 = mybir.dt.float16
i16 = mybir.dt.int16

_compiled = None


def build_program(n_hg=N_HG):
    nc = bacc.Bacc("TRN2", target_bir_lowering=False, debug=False,
                   num_devices=N_CORES)
    fl = nc.dram_tensor("feat_l", [H * C, W], f16, kind="ExternalInput").ap()
    fr = nc.dram_tensor("feat_r", [H * C, RW], f16,
                        kind="ExternalInput").ap()
    on = nc.dram_tensor("ones8", [128, 256], f16, kind="ExternalInput").ap()
    out = nc.dram_tensor("cost", [D, H, W], f16, kind="ExternalOutput").ap()

    sub = mybir.AluOpType.subtract
    band = mybir.AluOpType.bitwise_and

    with tile.TileContext(nc) as tc:
        with (
            tc.tile_pool(name="const", bufs=1) as constp,
            tc.tile_pool(name="inp", bufs=3) as inp,
            tc.tile_pool(name="diffp", bufs=DIFF_BUFS) as diffp,
            tc.tile_pool(name="absp", bufs=2) as absp,
            tc.tile_pool(name="stgp", bufs=4) as stgp,
            tc.tile_pool(name="psum", bufs=PSUM_BUFS, space="PSUM") as psp,
        ):
            # 8 one-hot stationaries: on8[j][k, m] = 1 iff m == 4*j + k//32
            on8 = constp.tile([128, 256], f16, name="on8")
            nc.sync.dma_start(on8[:], on[:])
            on8v = on8[:].rearrange("p (j m) -> p j m", j=8)

            def emit_loads(g):
                l16 = inp.tile([128, W], f16, name="l16", tag="l16")
                nc.sync.dma_start(l16[:], fl[128 * g:128 * (g + 1), :])
                rst = inp.tile([128, RW], f16, name="rst", tag="rst")
                nc.sync.dma_start(rst[:], fr[128 * g:128 * (g + 1), :])
                return l16, rst

            def sub_aps(la, ra, lo, hi):
                n = hi - lo
                lpart = list(la.ap)[0]
                rpart = list(ra.ap)[0]
                in0 = AP(la.tensor, la.offset,
                         [lpart, [0, n], [0, 4], [1, W]])
                in1 = AP(ra.tensor, ra.offset + 12 - lo,
                         [rpart, [-1, n], [524, 4], [1, W]])
                return in0, in1

            def emit_compute(g, l16, rst):
                h0 = HG * g
                dif = diffp.tile([128, NQ * FQ], f16, name="dif")
                dif4 = dif[:].rearrange("p (q s w) -> p q s w", q=NQ, s=4)
                la, ra = l16[:], rst[:]

                # subtracts: DVE q in [0, SQ) chunked; GpSimd the rest
                for lo in range(0, SQ, SUB_CHUNK):
                    hi = min(lo + SUB_CHUNK, SQ)
                    in0, in1 = sub_aps(la, ra, lo, hi)
                    nc.vector.tensor_tensor(dif4[:, lo:hi], in0, in1, op=sub)
                if SQ < NQ:
                    in0, in1 = sub_aps(la, ra, SQ, NQ)
                    nc.gpsimd.tensor_tensor(dif4[:, SQ:NQ], in0, in1, op=sub)

                # |diff| split by q-range; ACT chunked so the PE can start on
                # low q's early. With OOP_ABS the ACT share lands in separate
                # tiles (reduces tile/port contention); DVE share stays
                # in-place (SBUF budget).
                dfl = dif[:]
                a0, a1 = ABS_ACT_Q * FQ, ABS_DVE_Q * FQ
                mm_src = {}            # q -> (ap, base elem offset of q)
                bounds = list(range(0, ABS_ACT_Q, ABS_CHUNK)) + [ABS_ACT_Q]
                for lo, hi in zip(bounds, bounds[1:]):
                    src = dfl[:, lo * FQ:hi * FQ]
                    if OOP_ABS:
                        ab = absp.tile([128, (hi - lo) * FQ], f16, name="ab",
                                       tag=f"ab{lo}")
                        dst = ab[:]
                    else:
                        dst = src
                    nc.scalar.activation(dst, src,
                                         mybir.ActivationFunctionType.Abs)
                    for q in range(lo, hi):
                        mm_src[q] = (dst, (q - lo) * FQ)
                # DVE/GpSimd abs: clear fp16 sign bit on an int16 view
                # (abs_max fails the walrus ISA check; this keeps 4x mode)
                if ABS_DVE_Q > ABS_ACT_Q:
                    dvi = dfl[:, a0:a1].bitcast(i16)
                    nc.vector.tensor_scalar(dvi, dvi, 0x7fff, None, op0=band)
                    for q in range(ABS_ACT_Q, ABS_DVE_Q):
                        mm_src[q] = (dfl, q * FQ)
                if ABS_DVE_Q < NQ:
                    pvi = dfl[:, a1:].bitcast(i16)
                    nc.gpsimd.tensor_scalar(pvi, pvi, 0x7fff, None, op0=band)
                    for q in range(ABS_DVE_Q, NQ):
                        mm_src[q] = (dfl, q * FQ)

                # channel reduce: one matmul per disparity. d = 24*cb + dd,
                # PSUM rows 4*dd + h (8 matmuls accumulate per 32-row block).
                pt = psp.tile([128, 1024], f32, name="pt")
                for d_ in range(D):
                    cb, dd = d_ // 24, d_ % 24
                    blk, j = dd // 8, dd % 8
                    q, s = d_ // 4, d_ % 4
                    ap, base = mm_src[q]
                    nc.tensor.matmul(
                        pt[32 * blk:32 * blk + 32, 512 * cb:512 * cb + 512],
                        on8v[:, j, :],
                        ap[:, base + s * W:base + s * W + W],
                        start=(j == 0), stop=(j == 7))

                # drain PSUM -> SBUF fp16
                stg = stgp.tile([128, 1024], f16, name="stg")
                nc.scalar.copy(stg[0:96, :], pt[0:96, :])

                # out DMA: stg row 4*dd + h, col block cb -> out[24cb+dd, h0+h]
                for cb in range(2):
                    nc.sync.dma_start(
                        out[24 * cb:24 * cb + 24, h0:h0 + HG, :],
                        stg[0:96, 512 * cb:512 * cb + 512])

            q0 = emit_loads(0)
            q1 = emit_loads(1) if n_hg > 1 else None
            for g in range(n_hg):
                nxt = emit_loads(g + 2) if g + 2 < n_hg else None
                emit_compute(g, *q0)
                q0, q1 = q1, nxt
    nc.compile()
    return nc


def prep_in_maps(feat_l, feat_r):
    on = np.zeros((128, 8, 32), np.float16)
    for k in range(128):
        for j in range(8):
            on[k, j, 4 * j + k // 32] = 1.0
    on = on.reshape(128, 256)

    lt = np.ascontiguousarray(feat_l.transpose(0, 2, 1, 3)) \
        .reshape(B, H * C, W).astype(np.float16)

    rt = np.ascontiguousarray(feat_r.transpose(0, 2, 1, 3)) \
        .reshape(B, H * C, W4).astype(np.float16)
    rs = np.zeros((B, H * C, RW), np.float16)
    # col 524*s + 12 + m = r[4m - s]; valid when m >= 1, or (m == 0 and s == 0)
    rs[:, :, 12:12 + W] = rt[:, :, 0::4]                      # s = 0
    for s in (1, 2, 3):
        vals = rt[:, :, 4 - s::4][:, :, :W - 1]               # m = 1..511
        rs[:, :, 524 * s + 13:524 * s + 13 + (W - 1)] = vals

    maps = []
    for i in range(N_CORES):
        maps.append({"feat_l": lt[i], "feat_r": rs[i], "ones8": on})
    return maps


def kernel(feat_l, feat_r, maxdisp):
    global _compiled
    feat_l = np.asarray(feat_l, dtype=np.float32)
    feat_r = np.asarray(feat_r, dtype=np.float32)
    assert int(maxdisp) == D
    assert feat_l.shape == (B, C, H, W) and feat_r.shape == (B, C, H, W4)
    if _compiled is None:
        _compiled = build_program()
    in_maps = prep_in_maps(feat_l, feat_r)
    res = run_bass_kernel_spmd(_compiled, in_maps, list(range(N_CORES)))
    return np.stack(
        [res.results[i]["cost"].astype(np.float32) for i in range(N_CORES)],
        axis=0)


# revision 26
# speedup vs baseline: 1.0294x; 1.0294x over previous
"""Trainium2 Bass kernel for BuildVolume2d (stereo cost volume, L1 over channels).

cost[b, d, h, w] = sum_c |feat_l[b,c,h,w] - feat_r[b,c,h,4w-d]|   (feat_r zero-padded left)

Sharding: batch B=8 -> 8 NeuronCores (data parallel, one sample per core).

v3 design (per core, sample b; h-groups of 4 rows, partitions p = 32h + c):
  - Host pre-bakes fp16 layouts (no on-chip casts / phase splits):
      l16[(h c), w]                  = l[c,h,w]
      rstack[(h c), 524*s + 12 + m]  = r[c,h,4m-s]  (zero where 4m-s < 0 or m < 0)
    For d = 4q+s:  diff[p, q, s, w] = l16[p, w] - rstack[p, 524*s + 12 + w - q]
    i.e. in0 is a pure broadcast over (q, s) and in1 has q-stride -1: big
    multi-q subtracts are single instructions with a dense [128, n*4*512] out.
  - |x| in place, split across engines by q-range. Engine split tuned against
    the HW trace: the chip power-throttles (~0.7 util) and SBUF ports are
    contended, so ACT (1x but otherwise idle) carries most of the abs;
    DVE (4x tensor_scalar via int16 sign-bit clear) takes a little; GpSimd
    subtracts the tail q's.
  - Channel reduce: one matmul per disparity d, moving = dif[:, 512d:512d+512],
    8 one-hot stationaries on8[j][k, 4j + k//32]; 8 matmuls accumulate per
    32-row PSUM block; PSUM rows 4*dd + h, col block cb (d = 24cb + dd).
  - Drain: ACT copy PSUM[96,1024] f32 -> SBUF fp16.
  - Output DMA'd as fp16 [48,256,512] (2D sbuf -> 3D dram AP); host casts f32.
"""
import sys
sys.path.insert(0, '/opt/trn_rl_repo')

import numpy as np
import concourse.bass as bass
import concourse.tile as tile
from concourse import bacc, mybir
from concourse.ap import AP
from concourse.bass_utils import run_bass_kernel_spmd

# ---- problem constants (hardcoded per spec) ----
B, C, H, W = 8, 32, 256, 512
W4 = 4 * W
D = 48                     # maxdisp
N_CORES = 8
HG = 4                     # h rows per group
N_HG = H // HG             # 64
NQ = 12                    # d = 4q + s
RW = 2096                  # rstack row width (4 phase blocks of 524)
FQ = 4 * W                 # 2048 diff elems per q

# ---- engine split knobs (tuned against HW traces) ----
SQ = 8                     # DVE subtracts q in [0, SQ); GpSimd does [SQ, 12)
ABS_ACT_Q = 11             # ACT abs on q in [0, ABS_ACT_Q)
ABS_DVE_Q = 12             # DVE abs on [ABS_ACT_Q, ABS_DVE_Q); GpSimd rest
SUB_CHUNK = 3              # DVE sub emitted in q-chunks of this size
ABS_CHUNK = 2              # ACT abs emitted in q-chunks of this size
DIFF_BUFS = 3
PSUM_BUFS = 4
OOP_ABS = False            # ACT abs writes separate tiles (less tile contention)

f32 = mybir.dt.float32
f16 = mybir.dt.float16
i16 = mybir.dt.int16
f8 = mybir.dt.float8e4

_compiled = None


def build_program(n_hg=N_HG):
    nc = bacc.Bacc("TRN2", target_bir_lowering=False, debug=False,
                   num_devices=N_CORES)
    fl = nc.dram_tensor("feat_l", [H * C, W], f16, kind="ExternalInput").ap()
    fr = nc.dram_tensor("feat_r", [H * C, RW], f16,
                        kind="ExternalInput").ap()
    on = nc.dram_tensor("ones8", [128, 256], f16, kind="ExternalInput").ap()
    on8f = nc.dram_tensor("ones8f8", [128, 256], f8, kind="ExternalInput").ap()
    out = nc.dram_tensor("cost", [D, H, W], f16, kind="ExternalOutput").ap()

    sub = mybir.AluOpType.subtract
    band = mybir.AluOpType.bitwise_and

    with tile.TileContext(nc) as tc:
        with (
            tc.tile_pool(name="const", bufs=1) as constp,
            tc.tile_pool(name="inp", bufs=3) as inp,
            tc.tile_pool(name="diffp", bufs=DIFF_BUFS) as diffp,
            tc.tile_pool(name="absp", bufs=2) as absp,
            tc.tile_pool(name="stgp", bufs=4) as stgp,
            tc.tile_pool(name="psum", bufs=PSUM_BUFS, space="PSUM") as psp,
        ):
            # 8 one-hot stationaries: on8[j][k, m] = 1 iff m == 4*j + k//32
            on8 = constp.tile([128, 256], f16, name="on8")
            nc.sync.dma_start(on8[:], on[:])
            on8v = on8[:].rearrange("p (j m) -> p j m", j=8)
            on8_8 = constp.tile([128, 256], f8, name="on8_8")
            nc.sync.dma_start(on8_8[:], on8f[:])
            on8v8 = on8_8[:].rearrange("p (j m) -> p j m", j=8)

            def emit_loads(g):
                l16 = inp.tile([128, W], f16, name="l16", tag="l16")
                nc.sync.dma_start(l16[:], fl[128 * g:128 * (g + 1), :])
                rst = inp.tile([128, RW], f16, name="rst", tag="rst")
                nc.sync.dma_start(rst[:], fr[128 * g:128 * (g + 1), :])
                return l16, rst

            def sub_aps(la, ra, lo, hi):
                n = hi - lo
                lpart = list(la.ap)[0]
                rpart = list(ra.ap)[0]
                in0 = AP(la.tensor, la.offset,
                         [lpart, [0, n], [0, 4], [1, W]])
                in1 = AP(ra.tensor, ra.offset + 12 - lo,
                         [rpart, [-1, n], [524, 4], [1, W]])
                return in0, in1

            def emit_compute(g, l16, rst):
                h0 = HG * g
                dif = diffp.tile([128, NQ * FQ], f16, name="dif")
                dif4 = dif[:].rearrange("p (q s w) -> p q s w", q=NQ, s=4)
                la, ra = l16[:], rst[:]

                # subtracts: DVE q in [0, SQ) chunked; GpSimd the rest
                for lo in range(0, SQ, SUB_CHUNK):
                    hi = min(lo + SUB_CHUNK, SQ)
                    in0, in1 = sub_aps(la, ra, lo, hi)
                    nc.vector.tensor_tensor(dif4[:, lo:hi], in0, in1, op=sub)
                lpart, rpart = list(la.ap)[0], list(ra.ap)[0]
                for q in range(SQ, NQ):
                    in0q = AP(la.tensor, la.offset, [lpart, [0, 4], [1, W]])
                    in1q = AP(ra.tensor, ra.offset + 12 - q,
                              [rpart, [524, 4], [1, W]])
                    nc.gpsimd.tensor_tensor(dif4[:, q], in0q, in1q, op=sub)

                # |diff| split by q-range; ACT chunked so the PE can start on
                # low q's early. With OOP_ABS the ACT share lands in separate
                # tiles (reduces tile/port contention); DVE share stays
                # in-place (SBUF budget).
                dfl = dif[:]
                a0, a1 = ABS_ACT_Q * FQ, ABS_DVE_Q * FQ
                mm_src = {}            # q -> (ap, base elem offset of q)
                bounds = list(range(0, ABS_ACT_Q, ABS_CHUNK)) + [ABS_ACT_Q]
                for lo, hi in zip(bounds, bounds[1:]):
                    src = dfl[:, lo * FQ:hi * FQ]
                    if OOP_ABS:
                        ab = absp.tile([128, (hi - lo) * FQ], f16, name="ab",
                                       tag=f"ab{lo}")
                        dst = ab[:]
                    else:
                        dst = src
                    nc.scalar.activation(dst, src,
                                         mybir.ActivationFunctionType.Abs)
                    for q in range(lo, hi):
                        mm_src[q] = (dst, (q - lo) * FQ)
                # DVE/GpSimd abs: clear fp16 sign bit on an int16 view
                # (abs_max fails the walrus ISA check; this keeps 4x mode)
                if ABS_DVE_Q > ABS_ACT_Q:
                    dvi = dfl[:, a0:a1].bitcast(i16)
                    nc.vector.tensor_scalar(dvi, dvi, 0x7fff, None, op0=band)
                    for q in range(ABS_ACT_Q, ABS_DVE_Q):
                        mm_src[q] = (dfl, q * FQ)
                if ABS_DVE_Q < NQ:
                    pvi = dfl[:, a1:].bitcast(i16)
                    nc.gpsimd.tensor_scalar(pvi, pvi, 0x7fff, None, op0=band)
                    for q in range(ABS_DVE_Q, NQ):
                        mm_src[q] = (dfl, q * FQ)

                # channel reduce: one matmul per disparity. d = 24*cb + dd,
                # PSUM rows 4*dd + h (8 matmuls accumulate per 32-row block).
                pt = psp.tile([128, 1024], f32, name="pt")
                for d_ in range(D):
                    cb, dd = d_ // 24, d_ % 24
                    blk, j = dd // 8, dd % 8
                    q, s = d_ // 4, d_ % 4
                    ap, base = mm_src[q]
                    st = on8v8 if ap.dtype == f8 else on8v
                    nc.tensor.matmul(
                        pt[32 * blk:32 * blk + 32, 512 * cb:512 * cb + 512],
                        st[:, j, :],
                        ap[:, base + s * W:base + s * W + W],
                        start=(j == 0), stop=(j == 7))

                # drain PSUM -> SBUF fp16
                stg = stgp.tile([128, 1024], f16, name="stg")
                nc.scalar.copy(stg[0:96, :], pt[0:96, :])

                # out DMA: stg row 4*dd + h, col block cb -> out[24cb+dd, h0+h]
                for cb in range(2):
                    nc.sync.dma_start(
                        out[24 * cb:24 * cb + 24, h0:h0 + HG, :],
                        stg[0:96, 512 * cb:512 * cb + 512])

            q0 = emit_loads(0)
            q1 = emit_loads(1) if n_hg > 1 else None
            for g in range(n_hg):
                nxt = emit_loads(g + 2) if g + 2 < n_hg else None
                emit_compute(g, *q0)
                q0, q1 = q1, nxt
    nc.compile()
    return nc


def prep_in_maps(feat_l, feat_r):
    on = np.zeros((128, 8, 32), np.float16)
    for k in range(128):
        for j in range(8):
            on[k, j, 4 * j + k // 32] = 1.0
    on = on.reshape(128, 256)

    lt = np.ascontiguousarray(feat_l.transpose(0, 2, 1, 3)) \
        .reshape(B, H * C, W).astype(np.float16)

    rt = np.ascontiguousarray(feat_r.transpose(0, 2, 1, 3)) \
        .reshape(B, H * C, W4).astype(np.float16)
    rs = np.zeros((B, H * C, RW), np.float16)
    # col 524*s + 12 + m = r[4m - s]; valid when m >= 1, or (m == 0 and s == 0)
    rs[:, :, 12:12 + W] = rt[:, :, 0::4]                      # s = 0
    for s in (1, 2, 3):
        vals = rt[:, :, 4 - s::4][:, :, :W - 1]               # m = 1..511
        rs[:, :, 524 * s + 13:524 * s + 13 + (W - 1)] = vals

    import ml_dtypes
    on8f = on.astype(ml_dtypes.float8_e4m3fn)
    maps = []
    for i in range(N_CORES):
        maps.append({"feat_l": lt[i], "feat_r": rs[i], "ones8": on,
                     "ones8f8": on8f})
    return maps


def kernel(feat_l, feat_r, maxdisp):
    global _compiled
    feat_l = np.asarray(feat_l, dtype=np.float32)
    feat_r = np.asarray(feat_r, dtype=np.float32)
    assert int(maxdisp) == D
    assert feat_l.shape == (B, C, H, W) and feat_r.shape == (B, C, H, W4)
    if _compiled is None:
        _compiled = build_program()
    in_maps = prep_in_maps(feat_l, feat_r)
    res = run_bass_kernel_spmd(_compiled, in_maps, list(range(N_CORES)))
    return np.stack(
        [res.results[i]["cost"].astype(np.float32) for i in range(N_CORES)],
        axis=0)
